# revision 17
# baseline (speedup 1.0000x reference)
"""Trainium2 Bass kernel for the 2D-LSTM (nn_Lstm2D) problem.

Reference computation (B=64, C=3, H=W=128, P=4 patch, NC=512 cells):
  - image is cut into a 32x32 grid of 4x4 patches, raster-scanned (1024 steps)
  - per step t=(i,j):  gates = [x_t, h_prevrow_j] @ W_ih.T + h_{t-1} @ W_hh.T + b
                       i,f,g,o = split(gates); c = sig(f)*c + sig(i)*tanh(g)
                       h = sig(o)*tanh(c)
  - output: h at every grid cell -> (B, 512, 32, 32)

Strategy (8 NeuronCores, data-parallel over batch, 8 batch elements/core):
  - per-row "PRE" (bias + x@Wx.T + prevrow@Wp.T) is accumulated directly in
    PSUM, banded over 4 groups of 8 steps; the bias rides along as a ones-row
    appended to x.  The sequential recurrence matmuls (h @ W_hh.T) accumulate
    on top of the same PSUM regions (start=False), so the LSTM sigmoid reads
    finished gate values straight out of PSUM -- no adds or copies.
  - gates live in three single-bank PSUM tiles per band set: {g,i} / {f} / {o},
    double-banked across consecutive bands.  The matmul sweep runs in gate
    order g,i,f,o so each sigmoid can fire as soon as its bank is done; only
    sigma(o), tanh(c), and the h-mul trail the sweep.
  - tanh(g) is folded into the one sigmoid call: g-gate weights are pre-scaled
    by 2 host-side and tanh(g) = 2*sigmoid(2g) - 1 is recovered with fused
    scalar_tensor_tensor ops.
  - PRE matmuls for the *next* band (including next row's band 0, which only
    needs the first 8 columns of this row's h) are interleaved into TensorE
    idle gaps, two slots per step.
  - h row buffers ping-pong (2-row unrolled hardware loop); h is produced
    directly in bf16 and DMA'd out as bf16 (converted to f32 host-side).
"""

import numpy as np
import ml_dtypes

B = 64
C = 3
H = W = 128
P = 4
NCELL = 512
IN = C * P * P           # 48
IN1 = IN + 1             # +1 ones-row carrying the bias
KPAD = 128               # x contraction zero-padded to full partition dim
SY = SX = 32
NCORES = 8
BL = B // NCORES         # 8 batch elements per core
KC = NCELL // 128        # 4 contraction chunks for h
MC = (4 * NCELL) // 128  # 16 gate-dim chunks
NBAND = 4                # 4 bands of 8 steps per row
BSTEP = SX // NBAND      # 8 steps per band
# slot order (m-chunks of 128 gate rows): f0..f3, g0..g3, i0..i3, o0..o3
# sweep runs f first (sig(f) and f*c complete early), o last (shortest
# dependent chain: h = sig(o)*tanh(c)).
SLOT_TO_MCHUNK = [4, 5, 6, 7, 8, 9, 10, 11, 0, 1, 2, 3, 12, 13, 14, 15]

BF16 = ml_dtypes.bfloat16


def _build_module(sy=SY):
    import concourse.bass as bass
    import concourse.bacc as bacc
    import concourse.tile as tile
    import concourse.mybir as mybir

    f32 = mybir.dt.float32
    bf16 = mybir.dt.bfloat16
    SIG = mybir.ActivationFunctionType.Sigmoid
    TANH = mybir.ActivationFunctionType.Tanh
    SUB = mybir.AluOpType.subtract
    MULT = mybir.AluOpType.mult
    ADD = mybir.AluOpType.add

    nc = bacc.Bacc()

    # x rows padded by one (prefetch of row sy reads harmless zeros)
    x_d = nc.declare_dram_parameter("xt", [KPAD, sy + 1, SX * BL], bf16,
                                    isOutput=False)
    whh_d = nc.declare_dram_parameter("whht", [128, KC * MC * 128], bf16,
                                      isOutput=False)
    wp_d = nc.declare_dram_parameter("wpt", [128, KC * MC * 128], bf16,
                                     isOutput=False)
    wx_d = nc.declare_dram_parameter("wxt", [KPAD, MC * 128], bf16,
                                     isOutput=False)
    out_d = nc.declare_dram_parameter("out", [128, KC, sy * SX, BL], bf16,
                                      isOutput=True)

    with tile.TileContext(nc) as tc:
        with (
            tc.tile_pool(name="persist", bufs=1) as persist,
            tc.tile_pool(name="acts_f", bufs=3) as afpool,
            tc.tile_pool(name="acts_gi", bufs=3) as agpool,
            tc.tile_pool(name="acts_o", bufs=3) as aopool,
            tc.tile_pool(name="tmp", bufs=6) as tpool,
        ):
            whh_sb = persist.tile([128, KC, MC, 128], bf16)
            wp_sb = persist.tile([128, KC, MC, 128], bf16)
            wx_sb = persist.tile([KPAD, MC, 128], bf16)
            c_sb = persist.tile([128, KC, BL], f32)
            hbfA = persist.tile([128, KC, SX, BL], bf16)
            hbfB = persist.tile([128, KC, SX, BL], bf16)
            xrA = persist.tile([KPAD, SX * BL], bf16)
            xrB = persist.tile([KPAD, SX * BL], bf16)

            # gate PSUM banks: {g,i} (8 slots), {f} (4), {o} (4), x2 band sets.
            # Placed at fixed banks, each with a same-address READ ALIAS used
            # by the sigmoids: the alias hides the matmul->sigmoid dependency
            # from the framework's per-instruction semaphore chain (64 matmul
            # incs serialize at ~35ns and would gate the sigmoids ~0.5us
            # late); the dependency is enforced instead by one manual
            # semaphore inc on each gate-group's last matmul.
            def pl(name, nslot, bank):
                return nc.place_psum_tensor(
                    name, [128, nslot, BSTEP, BL], f32, bank)

            GI = [pl("gi0", 8, 0), pl("gi1", 8, 1)]
            GIR = [pl("gi0r", 8, 0), pl("gi1r", 8, 1)]
            PF = [pl("pf0", 4, 2), pl("pf1", 4, 3)]
            PFR = [pl("pf0r", 4, 2), pl("pf1r", 4, 3)]
            PO = [pl("po0", 4, 4), pl("po1", 4, 5)]
            POR = [pl("po0r", 4, 4), pl("po1r", 4, 5)]

            nc.sync.dma_start(out=whh_sb[:], in_=whh_d[:])
            nc.sync.dma_start(out=wp_sb[:], in_=wp_d[:])
            nc.sync.dma_start(out=wx_sb[:], in_=wx_d[:])
            nc.vector.memset(c_sb[:], 0.0)
            nc.vector.memset(hbfA[:], 0.0)
            nc.vector.memset(hbfB[:], 0.0)

            # pull the sigmoid/tanh ACT table load out of the row loop
            warm = persist.tile([1, 1], f32)
            nc.vector.memset(warm[:], 0.0)
            nc.scalar.activation(out=warm[:], in_=warm[:], func=SIG)
            nc.scalar.activation(out=warm[:], in_=warm[:], func=TANH)

            def psum_dst(s, band_set):
                if s < 4:
                    return PF[band_set][:, s, :, :]
                if s < 12:
                    return GI[band_set][:, s - 4, :, :]
                return PO[band_set][:, s - 12, :, :]

            def psum_dst_j(s, jl, band_set):
                if s < 4:
                    return PF[band_set][:, s, jl, :]
                if s < 12:
                    return GI[band_set][:, s - 4, jl, :]
                return PO[band_set][:, s - 12, jl, :]

            def pre_slot(s, band, band_set, xr, hsrc):
                # bias + x @ Wx.T (+ prevrow @ Wp.T) for 8 steps of one slot
                dst = psum_dst(s, band_set)
                first = s in (0, 4, 12)  # first matmul into this PSUM bank
                nc.tensor.matmul(
                    dst, wx_sb[:, s, :],
                    xr[:, band * BSTEP * BL:(band + 1) * BSTEP * BL],
                    start=first, stop=False, skip_group_check=True)
                for k in range(KC):
                    nc.tensor.matmul(
                        dst, wp_sb[:, k, s, :],
                        hsrc[:, k, band * BSTEP:(band + 1) * BSTEP, :],
                        start=False, stop=False, skip_group_check=True)

            def emit_step(j, cur, prev, xr_cur, xr_nxt):
                band_set = (j // BSTEP) % 2
                jl = j % BSTEP

                def rhs_h(k):
                    if j == 0:
                        return prev[:, k, SX - 1, :]
                    return cur[:, k, j - 1, :]

                def hidden_dst(s):
                    # same PSUM bytes via the untracked alias tensor
                    if s < 4:
                        return PFR[band_set][:, s, jl, :]
                    if s < 12:
                        return GIR[band_set][:, s - 4, jl, :]
                    return POR[band_set][:, s - 12, jl, :]

                def sweep(s0, s1):
                    # Only the group's LAST matmul writes the tracked tensor;
                    # the rest write the same-address alias so they carry no
                    # semaphore updates (matmuls complete in pc order, so the
                    # last one's inc implies the group).  This keeps the
                    # engine's serialized sem-inc conveyor (~35ns each) short.
                    for s in range(s0, s1):
                        for k in range(KC):
                            last = s == s1 - 1 and k == KC - 1
                            dst = (psum_dst_j(s, jl, band_set) if last
                                   else hidden_dst(s))
                            nc.tensor.matmul(
                                dst, whh_sb[:, k, s, :], rhs_h(k),
                                start=False, stop=(k == KC - 1),
                                skip_group_check=True)

                # high_priority pins the sweep ahead of interleaved PRE work
                # in the scheduler's per-engine streams (dep-readiness still
                # keeps PRE before the band's first consuming step), so PRE
                # fills real TensorE idle gaps instead of landing mid-sweep
                # and pushing the sigmoids' sem thresholds deeper into the
                # serialized inc conveyor.
                with tc.high_priority(offset=150):
                    sweep(0, 4)     # f
                    acts_f = afpool.tile([128, 4, BL], f32)
                    nc.scalar.activation(
                        out=acts_f[:], in_=PF[band_set][:, :, jl, :],
                        func=SIG)
                    fc = tpool.tile([128, KC, BL], f32)
                    nc.vector.tensor_mul(fc[:], acts_f[:], c_sb[:])
                    sweep(4, 12)    # g, i
                    acts_gi = agpool.tile([128, 8, BL], f32)
                    nc.scalar.activation(
                        out=acts_gi[:], in_=GI[band_set][:, :, jl, :],
                        func=SIG)
                    # t2 = (sig(2g) - 0.5) * sig(i) = tanh(g)/2 * sig(i)
                    t2 = tpool.tile([128, KC, BL], f32)
                    nc.vector.scalar_tensor_tensor(
                        out=t2[:], in0=acts_gi[:, 0:4, :], scalar=0.5,
                        in1=acts_gi[:, 4:8, :], op0=SUB, op1=MULT)
                    # c = 2*t2 + f*c
                    nc.vector.scalar_tensor_tensor(
                        out=c_sb[:], in0=t2[:], scalar=2.0, in1=fc[:],
                        op0=MULT, op1=ADD)
                    sweep(12, 16)   # o
                    acts_o = aopool.tile([128, 4, BL], f32)
                    nc.scalar.activation(
                        out=acts_o[:], in_=PO[band_set][:, :, jl, :],
                        func=SIG)
                    tc_t = tpool.tile([128, KC, BL], f32)
                    nc.scalar.activation(out=tc_t[:], in_=c_sb[:], func=TANH)
                    nc.vector.tensor_mul(cur[:, :, j, :], acts_o[:], tc_t[:])

                # interleave 2 PRE slots of the next band into the gap
                band_next = j // BSTEP + 1
                s0 = 2 * (j % BSTEP)
                if band_next < NBAND:
                    for s in (s0, s0 + 1):
                        pre_slot(s, band_next, band_next % 2, xr_cur, prev)
                else:  # next row's band 0 (needs this row's h cols 0..7)
                    for s in (s0, s0 + 1):
                        pre_slot(s, 0, 0, xr_nxt, cur)

            def row_section(cur, prev, xr_cur, xr_nxt, row_expr):
                # prefetch next row's x (row sy reads the zero padding)
                nc.gpsimd.dma_start(out=xr_nxt[:],
                                    in_=x_d[:, bass.ds(row_expr + 1, 1), :])
                for j in range(SX):
                    emit_step(j, cur, prev, xr_cur, xr_nxt)
                nc.gpsimd.dma_start(
                    out=out_d[:, :, bass.ds(row_expr * SX, SX), :],
                    in_=cur[:])

            # row 0: x + band 0
            nc.gpsimd.dma_start(out=xrA[:], in_=x_d[:, 0, :])
            for s in range(MC):
                pre_slot(s, 0, 0, xrA, hbfB)

            with tc.For_i(0, sy // 2) as iv:
                row_section(hbfA, hbfB, xrA, xrB, iv * 2)
                row_section(hbfB, hbfA, xrB, xrA, iv * 2 + 1)

    nc.compile()
    return nc


_CACHE = {}


def _get_module(sy=SY):
    if sy not in _CACHE:
        _CACHE[sy] = _build_module(sy)
    return _CACHE[sy]


def _prep_shared(W_ih, W_hh, b_ih, b_hh):
    perm = np.array(SLOT_TO_MCHUNK)
    scale = np.ones((16, 1), np.float32)
    scale[8:12] = 2.0  # g-gate rows pre-scaled: tanh(g) = 2*sig(2g) - 1

    wih_t = np.ascontiguousarray(W_ih.T.astype(np.float32))     # (560, 2048)
    wih_t = (wih_t.reshape(560, 16, 128) * scale[None]).astype(np.float32)
    wih_t = wih_t[:, perm, :]                                   # slot order
    bias = ((b_ih + b_hh).astype(np.float32).reshape(16, 128) * scale)[perm]
    wx = np.zeros((KPAD, 16, 128), np.float32)
    wx[:IN] = wih_t[:IN]
    wx[IN] = bias
    wx = wx.reshape(KPAD, MC * 128)
    wp = wih_t[IN:]                                             # (512,16,128)
    wp = wp.reshape(KC, 128, MC, 128).transpose(1, 0, 2, 3)
    wp = wp.reshape(128, KC * MC * 128)
    whh = np.ascontiguousarray(W_hh.T.astype(np.float32))       # (512, 2048)
    whh = (whh.reshape(512, 16, 128) * scale[None])[:, perm, :]
    whh = whh.reshape(KC, 128, MC, 128).transpose(1, 0, 2, 3)
    whh = whh.reshape(128, KC * MC * 128)
    return wx.astype(BF16), wp.astype(BF16), whh.astype(BF16)


def _prep_x(batch, sy=SY):
    # xs[i, j, b, :] = patch (C,P,P) flattened, matching the reference
    xs = batch.reshape(B, C, sy, P, SX, P).transpose(2, 4, 0, 1, 3, 5)
    xs = xs.reshape(sy, SX, B, IN)
    per_core = []
    for c in range(NCORES):
        xc = xs[:, :, c * BL:(c + 1) * BL, :]          # (sy, SX, BL, IN)
        xc = xc.transpose(3, 0, 1, 2).reshape(IN, sy, SX * BL)
        xa = np.zeros((KPAD, sy + 1, SX * BL), np.float32)
        xa[:IN, :sy] = xc
        xa[IN, :, :] = 1.0                             # bias ones-row
        per_core.append(xa.astype(BF16))
    return per_core


def _run(batch, W_ih, W_hh, b_ih, b_hh, trace=False):
    from concourse.bass_utils import run_bass_kernel_spmd

    batch = np.asarray(batch, dtype=np.float32)
    wx, wp, whh = _prep_shared(
        np.asarray(W_ih), np.asarray(W_hh), np.asarray(b_ih), np.asarray(b_hh))
    xs = _prep_x(batch)

    nc = _get_module()
    in_maps = [
        {"xt": xs[c], "whht": whh, "wpt": wp, "wxt": wx}
        for c in range(NCORES)
    ]
    res = run_bass_kernel_spmd(nc, in_maps, list(range(NCORES)), trace=trace)

    outs = []
    for c in range(NCORES):
        arr = res.results[c]["out"].astype(np.float32)  # (128, KC, T, BL)
        # reference's to_image is a raw reshape of (B, T, NC) into
        # (B, NC, SY, SX): arr axes (BL, T, KC, 128) flatten to (BL, T*NC).
        arr = arr.transpose(3, 2, 1, 0).reshape(BL, NCELL, SY, SX)
        outs.append(arr)
    return np.concatenate(outs, axis=0).astype(np.float32), res


def kernel(batch, W_ih, W_hh, b_ih, b_hh):
    out, _ = _run(batch, W_ih, W_hh, b_ih, b_hh)
    return out


# revision 18
# speedup vs baseline: 1.0132x; 1.0132x over previous
"""Trainium2 Bass kernel for the 2D-LSTM (nn_Lstm2D) problem.

Reference computation (B=64, C=3, H=W=128, P=4 patch, NC=512 cells):
  - image is cut into a 32x32 grid of 4x4 patches, raster-scanned (1024 steps)
  - per step t=(i,j):  gates = [x_t, h_prevrow_j] @ W_ih.T + h_{t-1} @ W_hh.T + b
                       i,f,g,o = split(gates); c = sig(f)*c + sig(i)*tanh(g)
                       h = sig(o)*tanh(c)
  - output: h at every grid cell -> (B, 512, 32, 32)

Strategy (8 NeuronCores, data-parallel over batch, 8 batch elements/core):
  - per-row "PRE" (bias + x@Wx.T + prevrow@Wp.T) is accumulated directly in
    PSUM, banded over 4 groups of 8 steps; the bias rides along as a ones-row
    appended to x.  The sequential recurrence matmuls (h @ W_hh.T) accumulate
    on top of the same PSUM regions (start=False), so the LSTM sigmoid reads
    finished gate values straight out of PSUM -- no adds or copies.
  - gates live in three single-bank PSUM tiles per band set: {g,i} / {f} / {o},
    double-banked across consecutive bands.  The matmul sweep runs in gate
    order g,i,f,o so each sigmoid can fire as soon as its bank is done; only
    sigma(o), tanh(c), and the h-mul trail the sweep.
  - tanh(g) is folded into the one sigmoid call: g-gate weights are pre-scaled
    by 2 host-side and tanh(g) = 2*sigmoid(2g) - 1 is recovered with fused
    scalar_tensor_tensor ops.
  - PRE matmuls for the *next* band (including next row's band 0, which only
    needs the first 8 columns of this row's h) are interleaved into TensorE
    idle gaps, two slots per step.
  - h row buffers ping-pong (2-row unrolled hardware loop); h is produced
    directly in bf16 and DMA'd out as bf16 (converted to f32 host-side).
"""

import numpy as np
import ml_dtypes

B = 64
C = 3
H = W = 128
P = 4
NCELL = 512
IN = C * P * P           # 48
IN1 = IN + 1             # +1 ones-row carrying the bias
KPAD = 128               # x contraction zero-padded to full partition dim
SY = SX = 32
NCORES = 8
BL = B // NCORES         # 8 batch elements per core
KC = NCELL // 128        # 4 contraction chunks for h
MC = (4 * NCELL) // 128  # 16 gate-dim chunks
NBAND = 4                # 4 bands of 8 steps per row
BSTEP = SX // NBAND      # 8 steps per band
# slot order (m-chunks of 128 gate rows): f0..f3, g0..g3, i0..i3, o0..o3
# sweep runs f first (sig(f) and f*c complete early), o last (shortest
# dependent chain: h = sig(o)*tanh(c)).
SLOT_TO_MCHUNK = [4, 5, 6, 7, 8, 9, 10, 11, 0, 1, 2, 3, 12, 13, 14, 15]

BF16 = ml_dtypes.bfloat16


def _build_module(sy=SY):
    import concourse.bass as bass
    import concourse.bacc as bacc
    import concourse.tile as tile
    import concourse.mybir as mybir

    f32 = mybir.dt.float32
    bf16 = mybir.dt.bfloat16
    SIG = mybir.ActivationFunctionType.Sigmoid
    TANH = mybir.ActivationFunctionType.Tanh
    SUB = mybir.AluOpType.subtract
    MULT = mybir.AluOpType.mult
    ADD = mybir.AluOpType.add

    nc = bacc.Bacc()

    # x rows padded by one (prefetch of row sy reads harmless zeros)
    x_d = nc.declare_dram_parameter("xt", [KPAD, sy + 1, SX * BL], bf16,
                                    isOutput=False)
    whh_d = nc.declare_dram_parameter("whht", [128, KC * MC * 128], bf16,
                                      isOutput=False)
    wp_d = nc.declare_dram_parameter("wpt", [128, KC * MC * 128], bf16,
                                     isOutput=False)
    wx_d = nc.declare_dram_parameter("wxt", [KPAD, MC * 128], bf16,
                                     isOutput=False)
    out_d = nc.declare_dram_parameter("out", [128, KC, sy * SX, BL], bf16,
                                      isOutput=True)

    with tile.TileContext(nc) as tc:
        with (
            tc.tile_pool(name="persist", bufs=1) as persist,
            tc.tile_pool(name="acts_f", bufs=3) as afpool,
            tc.tile_pool(name="acts_gi", bufs=3) as agpool,
            tc.tile_pool(name="acts_o", bufs=3) as aopool,
            tc.tile_pool(name="tmp", bufs=6) as tpool,
        ):
            whh_sb = persist.tile([128, KC, MC, 128], bf16)
            wp_sb = persist.tile([128, KC, MC, 128], bf16)
            wx_sb = persist.tile([KPAD, MC, 128], bf16)
            c_sb = persist.tile([128, KC, BL], f32)
            hbfA = persist.tile([128, KC, SX, BL], bf16)
            hbfB = persist.tile([128, KC, SX, BL], bf16)
            xrA = persist.tile([KPAD, SX * BL], bf16)
            xrB = persist.tile([KPAD, SX * BL], bf16)

            # gate PSUM banks: {g,i} (8 slots), {f} (4), {o} (4), x2 band sets.
            # Placed at fixed banks, each with a same-address READ ALIAS used
            # by the sigmoids: the alias hides the matmul->sigmoid dependency
            # from the framework's per-instruction semaphore chain (64 matmul
            # incs serialize at ~35ns and would gate the sigmoids ~0.5us
            # late); the dependency is enforced instead by one manual
            # semaphore inc on each gate-group's last matmul.
            def pl(name, nslot, bank):
                return nc.place_psum_tensor(
                    name, [128, nslot, BSTEP, BL], f32, bank)

            GI = [pl("gi0", 8, 0), pl("gi1", 8, 1)]
            GIR = [pl("gi0r", 8, 0), pl("gi1r", 8, 1)]
            PF = [pl("pf0", 4, 2), pl("pf1", 4, 3)]
            PFR = [pl("pf0r", 4, 2), pl("pf1r", 4, 3)]
            PO = [pl("po0", 4, 4), pl("po1", 4, 5)]
            POR = [pl("po0r", 4, 4), pl("po1r", 4, 5)]

            nc.sync.dma_start(out=whh_sb[:], in_=whh_d[:])
            nc.sync.dma_start(out=wp_sb[:], in_=wp_d[:])
            nc.sync.dma_start(out=wx_sb[:], in_=wx_d[:])
            nc.vector.memset(c_sb[:], 0.0)
            nc.vector.memset(hbfA[:], 0.0)
            nc.vector.memset(hbfB[:], 0.0)

            # pull the sigmoid/tanh ACT table load out of the row loop
            warm = persist.tile([1, 1], f32)
            nc.vector.memset(warm[:], 0.0)
            nc.scalar.activation(out=warm[:], in_=warm[:], func=SIG)
            nc.scalar.activation(out=warm[:], in_=warm[:], func=TANH)

            def psum_dst(s, band_set):
                if s < 4:
                    return PF[band_set][:, s, :, :]
                if s < 12:
                    return GI[band_set][:, s - 4, :, :]
                return PO[band_set][:, s - 12, :, :]

            def psum_dst_j(s, jl, band_set):
                if s < 4:
                    return PF[band_set][:, s, jl, :]
                if s < 12:
                    return GI[band_set][:, s - 4, jl, :]
                return PO[band_set][:, s - 12, jl, :]

            def pre_slot(s, band, band_set, xr, hsrc):
                # bias + x @ Wx.T (+ prevrow @ Wp.T) for 8 steps of one slot
                dst = psum_dst(s, band_set)
                first = s in (0, 4, 12)  # first matmul into this PSUM bank
                nc.tensor.matmul(
                    dst, wx_sb[:, s, :],
                    xr[:, band * BSTEP * BL:(band + 1) * BSTEP * BL],
                    start=first, stop=False, skip_group_check=True)
                for k in range(KC):
                    nc.tensor.matmul(
                        dst, wp_sb[:, k, s, :],
                        hsrc[:, k, band * BSTEP:(band + 1) * BSTEP, :],
                        start=False, stop=False, skip_group_check=True)

            def emit_step(j, cur, prev, xr_cur, xr_nxt):
                band_set = (j // BSTEP) % 2
                jl = j % BSTEP

                def rhs_h(k):
                    if j == 0:
                        return prev[:, k, SX - 1, :]
                    return cur[:, k, j - 1, :]

                def hidden_dst(s):
                    # same PSUM bytes via the untracked alias tensor
                    if s < 4:
                        return PFR[band_set][:, s, jl, :]
                    if s < 12:
                        return GIR[band_set][:, s - 4, jl, :]
                    return POR[band_set][:, s - 12, jl, :]

                def sweep(s0, s1):
                    # Only the group's LAST matmul writes the tracked tensor;
                    # the rest write the same-address alias so they carry no
                    # semaphore updates (matmuls complete in pc order, so the
                    # last one's inc implies the group).  This keeps the
                    # engine's serialized sem-inc conveyor (~35ns each) short.
                    for s in range(s0, s1):
                        for k in range(KC):
                            last = s == s1 - 1 and k == KC - 1
                            dst = (psum_dst_j(s, jl, band_set) if last
                                   else hidden_dst(s))
                            nc.tensor.matmul(
                                dst, whh_sb[:, k, s, :], rhs_h(k),
                                start=False, stop=(k == KC - 1),
                                skip_group_check=True)

                # high_priority pins the sweep ahead of interleaved PRE work
                # in the scheduler's per-engine streams (dep-readiness still
                # keeps PRE before the band's first consuming step), so PRE
                # fills real TensorE idle gaps instead of landing mid-sweep
                # and pushing the sigmoids' sem thresholds deeper into the
                # serialized inc conveyor.
                with tc.high_priority(offset=150):
                    sweep(0, 4)     # f
                    acts_f = afpool.tile([128, 4, BL], f32)
                    nc.scalar.activation(
                        out=acts_f[:], in_=PF[band_set][:, :, jl, :],
                        func=SIG)
                    fc = tpool.tile([128, KC, BL], f32)
                    nc.vector.tensor_mul(fc[:], acts_f[:], c_sb[:])
                    sweep(4, 12)    # g, i
                    acts_gi = agpool.tile([128, 8, BL], f32)
                    nc.scalar.activation(
                        out=acts_gi[:], in_=GI[band_set][:, :, jl, :],
                        func=SIG)
                    # t2 = (sig(2g) - 0.5) * sig(i) = tanh(g)/2 * sig(i)
                    t2 = tpool.tile([128, KC, BL], f32)
                    nc.vector.scalar_tensor_tensor(
                        out=t2[:], in0=acts_gi[:, 0:4, :], scalar=0.5,
                        in1=acts_gi[:, 4:8, :], op0=SUB, op1=MULT)
                    # c = 2*t2 + f*c
                    nc.vector.scalar_tensor_tensor(
                        out=c_sb[:], in0=t2[:], scalar=2.0, in1=fc[:],
                        op0=MULT, op1=ADD)
                    sweep(12, 16)   # o
                    acts_o = aopool.tile([128, 4, BL], f32)
                    nc.scalar.activation(
                        out=acts_o[:], in_=PO[band_set][:, :, jl, :],
                        func=SIG)
                    tc_t = tpool.tile([128, KC, BL], f32)
                    nc.scalar.activation(out=tc_t[:], in_=c_sb[:], func=TANH)
                    nc.vector.tensor_mul(cur[:, :, j, :], acts_o[:], tc_t[:])

                # interleave PRE slots of the next band into the gap,
                # front-loaded (3/3/3/3/3/1/0/0) so the band-boundary step's
                # sweep is not preceded by PRE incs in the sem conveyor
                band_next = j // BSTEP + 1
                g = j % BSTEP
                slots = ([3 * g, 3 * g + 1, 3 * g + 2] if g < 5
                         else [15] if g == 5 else [])
                if band_next < NBAND:
                    for s in slots:
                        pre_slot(s, band_next, band_next % 2, xr_cur, prev)
                else:  # next row's band 0 (needs this row's h cols 0..7)
                    for s in slots:
                        pre_slot(s, 0, 0, xr_nxt, cur)

            def row_section(cur, prev, xr_cur, xr_nxt, row_expr):
                # prefetch next row's x (row sy reads the zero padding)
                nc.gpsimd.dma_start(out=xr_nxt[:],
                                    in_=x_d[:, bass.ds(row_expr + 1, 1), :])
                for j in range(SX):
                    emit_step(j, cur, prev, xr_cur, xr_nxt)
                nc.gpsimd.dma_start(
                    out=out_d[:, :, bass.ds(row_expr * SX, SX), :],
                    in_=cur[:])

            # row 0: x + band 0
            nc.gpsimd.dma_start(out=xrA[:], in_=x_d[:, 0, :])
            for s in range(MC):
                pre_slot(s, 0, 0, xrA, hbfB)

            with tc.For_i(0, sy // 4) as iv:
                row_section(hbfA, hbfB, xrA, xrB, iv * 4)
                row_section(hbfB, hbfA, xrB, xrA, iv * 4 + 1)
                row_section(hbfA, hbfB, xrA, xrB, iv * 4 + 2)
                row_section(hbfB, hbfA, xrB, xrA, iv * 4 + 3)

    nc.compile()
    return nc


_CACHE = {}


def _get_module(sy=SY):
    if sy not in _CACHE:
        _CACHE[sy] = _build_module(sy)
    return _CACHE[sy]


def _prep_shared(W_ih, W_hh, b_ih, b_hh):
    perm = np.array(SLOT_TO_MCHUNK)
    scale = np.ones((16, 1), np.float32)
    scale[8:12] = 2.0  # g-gate rows pre-scaled: tanh(g) = 2*sig(2g) - 1

    wih_t = np.ascontiguousarray(W_ih.T.astype(np.float32))     # (560, 2048)
    wih_t = (wih_t.reshape(560, 16, 128) * scale[None]).astype(np.float32)
    wih_t = wih_t[:, perm, :]                                   # slot order
    bias = ((b_ih + b_hh).astype(np.float32).reshape(16, 128) * scale)[perm]
    wx = np.zeros((KPAD, 16, 128), np.float32)
    wx[:IN] = wih_t[:IN]
    wx[IN] = bias
    wx = wx.reshape(KPAD, MC * 128)
    wp = wih_t[IN:]                                             # (512,16,128)
    wp = wp.reshape(KC, 128, MC, 128).transpose(1, 0, 2, 3)
    wp = wp.reshape(128, KC * MC * 128)
    whh = np.ascontiguousarray(W_hh.T.astype(np.float32))       # (512, 2048)
    whh = (whh.reshape(512, 16, 128) * scale[None])[:, perm, :]
    whh = whh.reshape(KC, 128, MC, 128).transpose(1, 0, 2, 3)
    whh = whh.reshape(128, KC * MC * 128)
    return wx.astype(BF16), wp.astype(BF16), whh.astype(BF16)


def _prep_x(batch, sy=SY):
    # xs[i, j, b, :] = patch (C,P,P) flattened, matching the reference
    xs = batch.reshape(B, C, sy, P, SX, P).transpose(2, 4, 0, 1, 3, 5)
    xs = xs.reshape(sy, SX, B, IN)
    per_core = []
    for c in range(NCORES):
        xc = xs[:, :, c * BL:(c + 1) * BL, :]          # (sy, SX, BL, IN)
        xc = xc.transpose(3, 0, 1, 2).reshape(IN, sy, SX * BL)
        xa = np.zeros((KPAD, sy + 1, SX * BL), np.float32)
        xa[:IN, :sy] = xc
        xa[IN, :, :] = 1.0                             # bias ones-row
        per_core.append(xa.astype(BF16))
    return per_core


def _run(batch, W_ih, W_hh, b_ih, b_hh, trace=False):
    from concourse.bass_utils import run_bass_kernel_spmd

    batch = np.asarray(batch, dtype=np.float32)
    wx, wp, whh = _prep_shared(
        np.asarray(W_ih), np.asarray(W_hh), np.asarray(b_ih), np.asarray(b_hh))
    xs = _prep_x(batch)

    nc = _get_module()
    in_maps = [
        {"xt": xs[c], "whht": whh, "wpt": wp, "wxt": wx}
        for c in range(NCORES)
    ]
    res = run_bass_kernel_spmd(nc, in_maps, list(range(NCORES)), trace=trace)

    outs = []
    for c in range(NCORES):
        arr = res.results[c]["out"].astype(np.float32)  # (128, KC, T, BL)
        # reference's to_image is a raw reshape of (B, T, NC) into
        # (B, NC, SY, SX): arr axes (BL, T, KC, 128) flatten to (BL, T*NC).
        arr = arr.transpose(3, 2, 1, 0).reshape(BL, NCELL, SY, SX)
        outs.append(arr)
    return np.concatenate(outs, axis=0).astype(np.float32), res


def kernel(batch, W_ih, W_hh, b_ih, b_hh):
    out, _ = _run(batch, W_ih, W_hh, b_ih, b_hh)
    return out
